# revision 20
# baseline (speedup 1.0000x reference)
"""GCN aggregator kernel for Trainium2 (Bass/Tile), 8-core data-parallel.

Computes: out = relu(((sum_k neigh[:,k,:] + self) / (K+1)) @ W + b)
Sharding: nodes (N) split evenly across 8 NeuronCores; W/b replicated.

Per 128-node tile on each core:
  1. DMA neigh tile [128, K*D] + self tile [128, D]           (sync HWDGE)
  2. DVE reduce_sum over k (strided AP) + add self            (VectorE)
  3. PE transpose sum -> sumT in PSUM, ACT copy w/ 1/(K+1)    (TensorE/ScalarE)
  4. PE GEMM sumT.T @ W accumulated over 4 d-chunks + bias    (TensorE)
  5. ACT relu PSUM->SBUF, DMA store                           (ScalarE HWDGE)
"""

import os
import sys

import numpy as np

for _p in ("/opt/trn_rl_repo", "/root/.axon_site/_ro/trn_rl_repo"):
    if os.path.isdir(_p) and _p not in sys.path:
        sys.path.insert(0, _p)

import concourse.bass as bass
import concourse.tile as tile
from concourse import bacc, mybir
from concourse.masks import make_identity

N, K, D, O = 16384, 25, 512, 1024
N_CORES = 8
P = 128  # nodes per tile (partition count)
INV = 1.0 / (K + 1)
FP = mybir.dt.float32


def _tree_fold(nc, t, g):
    """In-place pairwise fold of `g` contiguous D-sized groups in tile t;
    result lands in t[:, :D]."""
    while g > 1:
        lo = g // 2
        nc.vector.tensor_add(
            t[:, : lo * D], t[:, : lo * D], t[:, (g - lo) * D : g * D]
        )
        g -= lo


def build_nc(n_nodes: int, neigh_bufs: int = 3) -> bass.Bass:
    """Build the per-core Bass program for a shard of `n_nodes` nodes."""
    assert n_nodes % P == 0
    nt = n_nodes // P

    nc = bacc.Bacc("TRN2", target_bir_lowering=False, debug=False)
    self_h = nc.dram_tensor("self_vecs", [n_nodes, D], FP, kind="ExternalInput")
    neigh_h = nc.dram_tensor("neigh_vecs", [n_nodes, K, D], FP, kind="ExternalInput")
    w_h = nc.dram_tensor("W", [D, O], FP, kind="ExternalInput")
    b_h = nc.dram_tensor("b", [O], FP, kind="ExternalInput")
    out_h = nc.dram_tensor("out", [n_nodes, O], FP, kind="ExternalOutput")

    n_dc = D // P  # d-chunks for the GEMM contraction

    with tile.TileContext(nc) as tc:
        with (
            tc.tile_pool(name="const", bufs=1) as const_pool,
            tc.tile_pool(name="neigh", bufs=neigh_bufs) as neigh_pool,
            tc.tile_pool(name="small", bufs=3) as small_pool,
            tc.tile_pool(name="outp", bufs=2) as out_pool,
            tc.tile_pool(name="ps_t", bufs=2, space="PSUM") as ps_t_pool,
            tc.tile_pool(name="ps_o", bufs=2, space="PSUM") as ps_o_pool,
        ):
            # --- constants (loaded once; the W/b DMAs are emitted after tile
            # 0's loads so the neigh stream starts immediately on the ring) ---
            # w_sb[p, c, o] = W[c*128 + p, o] -> chunk c is the rhs for d-chunk c
            w_sb = const_pool.tile([P, n_dc * O], FP)
            b_sb = const_pool.tile([1, O], FP)
            ident = const_pool.tile([P, P], FP)
            make_identity(nc, ident)
            ones = const_pool.tile([1, P], FP)
            nc.gpsimd.memset(ones, 1.0)

            def transpose_scaled(src, tag, ps_bufs):
                """PE-transpose src [n,d] into [d,n] chunks, scale by 1/(K+1)
                on the PSUM->SBUF copy."""
                tps = ps_t_pool.tile(
                    [P, D], FP, tag=f"tps_{tag}", name=f"tps_{tag}", bufs=ps_bufs
                )
                for c in range(n_dc):
                    nc.tensor.transpose(
                        tps[:, bass.ts(c, P)], src[:, bass.ts(c, P)], ident
                    )
                t_sb = small_pool.tile(
                    [P, D], FP, tag=f"tsb_{tag}", name=f"tsb_{tag}",
                    bufs=(1 if tag == "b" else None),
                )
                nc.scalar.activation(
                    t_sb, tps, mybir.ActivationFunctionType.Copy, scale=INV
                )
                return t_sb

            def gemm_acc(out_pss, sumT, start):
                for c in range(n_dc):
                    for oh in range(len(out_pss)):
                        nc.tensor.matmul(
                            out_pss[oh],
                            lhsT=sumT[:, bass.ts(c, P)],
                            rhs=w_sb[:, c * O + oh * 512 : c * O + oh * 512 + 512],
                            start=(start and c == 0),
                            stop=False,
                        )

            k1 = (K + 1) // 2  # 13
            k2 = K - k1  # 12
            for i in range(nt):
                # split the neigh load so the k-sum (DVE tree adds; these run
                # at model speed where tensor_reduce measured ~1.6x slower)
                # starts while the second half streams, and SBUF slots
                # release at half-tile granularity
                nh1 = neigh_pool.tile([P, k1 * D], FP, tag="nh1", name="nh1")
                nc.sync.dma_start(nh1, neigh_h[bass.ts(i, P), 0:k1, :])
                nh2 = neigh_pool.tile([P, k2 * D], FP, tag="nh2", name="nh2")
                nc.sync.dma_start(nh2, neigh_h[bass.ts(i, P), k1:K, :])
                self_t = small_pool.tile([P, D], FP)
                nc.sync.dma_start(self_t, self_h[bass.ts(i, P), :])
                if i == 0:
                    nc.sync.dma_start(
                        w_sb, w_h[:, :].rearrange("(c p) o -> p c o", p=P)
                    )
                    nc.sync.dma_start(b_sb, b_h[:])

                out_sb = out_pool.tile([P, O], FP)
                n_oh = O // 512
                out_pss = [
                    ps_o_pool.tile([P, 512], FP, tag=f"out_ps{oh}", name=f"out_ps{oh}")
                    for oh in range(n_oh)
                ]

                if i == nt - 1:
                    # tail tile: GEMM per half so half1's GEMM overlaps
                    # half2's DMA+tree and keeps the PE warm for half2's GEMM
                    _tree_fold(nc, nh1, k1)
                    summ1 = small_pool.tile([P, D], FP, tag="summ", name="summ1")
                    nc.vector.tensor_add(summ1, nh1[:, :D], self_t)
                    sumT1 = transpose_scaled(summ1, "a", None)
                    gemm_acc(out_pss, sumT1, start=True)
                    _tree_fold(nc, nh2, k2)
                    sumT2 = transpose_scaled(nh2[:, :D], "b", 1)
                    gemm_acc(out_pss, sumT2, start=False)
                else:
                    _tree_fold(nc, nh1, k1)
                    _tree_fold(nc, nh2, k2)
                    s12 = small_pool.tile([P, D], FP)
                    nc.vector.tensor_add(s12, nh1[:, :D], nh2[:, :D])
                    summ = small_pool.tile([P, D], FP)
                    nc.vector.tensor_add(summ, s12, self_t)
                    sumT = transpose_scaled(summ, "a", None)
                    gemm_acc(out_pss, sumT, start=True)

                for oh in range(n_oh):
                    # bias via K=1 matmul: ones.T @ b broadcasts b over nodes
                    nc.tensor.matmul(
                        out_pss[oh],
                        lhsT=ones,
                        rhs=b_sb[:, bass.ts(oh, 512)],
                        start=False,
                        stop=True,
                    )
                    nc.scalar.activation(
                        out_sb[:, bass.ts(oh, 512)],
                        out_pss[oh],
                        mybir.ActivationFunctionType.Relu,
                    )
                nc.scalar.dma_start(out_h[bass.ts(i, P), :], out_sb)

    nc.compile()
    return nc


def shard_inputs(inputs: dict) -> list[dict]:
    n = inputs["self_vecs"].shape[0]
    per = n // N_CORES
    maps = []
    for c in range(N_CORES):
        sl = slice(c * per, (c + 1) * per)
        maps.append(
            {
                "self_vecs": np.ascontiguousarray(inputs["self_vecs"][sl], np.float32),
                "neigh_vecs": np.ascontiguousarray(
                    inputs["neigh_vecs"][sl], np.float32
                ),
                "W": np.ascontiguousarray(inputs["W"], np.float32),
                "b": np.ascontiguousarray(inputs["b"], np.float32),
            }
        )
    return maps


def run_sharded(inputs: dict, trace: bool = False, **kwargs):
    from concourse.bass_utils import run_bass_kernel_spmd

    in_maps = shard_inputs(inputs)
    n_nodes = in_maps[0]["self_vecs"].shape[0]
    nc = build_nc(n_nodes)
    res = run_bass_kernel_spmd(
        nc, in_maps, core_ids=list(range(N_CORES)), trace=trace, **kwargs
    )
    out = np.concatenate([res.results[c]["out"] for c in range(N_CORES)], axis=0)
    return out, res


def kernel(**inputs) -> np.ndarray:
    out, _ = run_sharded(inputs, trace=False)
    return out


# revision 23
# speedup vs baseline: 1.0163x; 1.0163x over previous
"""GCN aggregator kernel for Trainium2 (Bass/Tile), 8-core data-parallel.

Computes: out = relu(((sum_k neigh[:,k,:] + self) / (K+1)) @ W + b)
Sharding: nodes (N) split evenly across 8 NeuronCores; W/b replicated.

Per 128-node tile on each core:
  1. DMA neigh tile [128, K*D] + self tile [128, D]           (sync HWDGE)
  2. DVE reduce_sum over k (strided AP) + add self            (VectorE)
  3. PE transpose sum -> sumT in PSUM, ACT copy w/ 1/(K+1)    (TensorE/ScalarE)
  4. PE GEMM sumT.T @ W accumulated over 4 d-chunks + bias    (TensorE)
  5. ACT relu PSUM->SBUF, DMA store                           (ScalarE HWDGE)
"""

import os
import sys

import numpy as np

for _p in ("/opt/trn_rl_repo", "/root/.axon_site/_ro/trn_rl_repo"):
    if os.path.isdir(_p) and _p not in sys.path:
        sys.path.insert(0, _p)

import concourse.bass as bass
import concourse.tile as tile
from concourse import bacc, mybir
from concourse.masks import make_identity

N, K, D, O = 16384, 25, 512, 1024
N_CORES = 8
P = 128  # nodes per tile (partition count)
INV = 1.0 / (K + 1)
FP = mybir.dt.float32


def _tree_fold(nc, t, g):
    """In-place pairwise fold of `g` contiguous D-sized groups in tile t;
    result lands in t[:, :D]."""
    while g > 1:
        lo = g // 2
        nc.vector.tensor_add(
            t[:, : lo * D], t[:, : lo * D], t[:, (g - lo) * D : g * D]
        )
        g -= lo


def build_nc(n_nodes: int, neigh_bufs: int = 3) -> bass.Bass:
    """Build the per-core Bass program for a shard of `n_nodes` nodes."""
    assert n_nodes % P == 0
    nt = n_nodes // P

    nc = bacc.Bacc("TRN2", target_bir_lowering=False, debug=False)
    self_h = nc.dram_tensor("self_vecs", [n_nodes, D], FP, kind="ExternalInput")
    neigh_h = nc.dram_tensor("neigh_vecs", [n_nodes, K, D], FP, kind="ExternalInput")
    w_h = nc.dram_tensor("W", [D, O], FP, kind="ExternalInput")
    b_h = nc.dram_tensor("b", [O], FP, kind="ExternalInput")
    out_h = nc.dram_tensor("out", [n_nodes, O], FP, kind="ExternalOutput")

    n_dc = D // P  # d-chunks for the GEMM contraction

    with tile.TileContext(nc) as tc:
        with (
            tc.tile_pool(name="const", bufs=1) as const_pool,
            tc.tile_pool(name="neigh", bufs=neigh_bufs) as neigh_pool,
            tc.tile_pool(name="small", bufs=3) as small_pool,
            tc.tile_pool(name="outp", bufs=2) as out_pool,
            tc.tile_pool(name="ps_t", bufs=2, space="PSUM") as ps_t_pool,
            tc.tile_pool(name="ps_o", bufs=2, space="PSUM") as ps_o_pool,
        ):
            # --- constants (loaded once) ---
            # w_sb[p, c, o] = W[c*128 + p, o] -> chunk c is the rhs for d-chunk c
            w_sb = const_pool.tile([P, n_dc * O], FP)
            nc.sync.dma_start(w_sb, w_h[:, :].rearrange("(c p) o -> p c o", p=P))
            b_sb = const_pool.tile([1, O], FP)
            nc.sync.dma_start(b_sb, b_h[:])
            ident = const_pool.tile([P, P], FP)
            make_identity(nc, ident)
            ones = const_pool.tile([1, P], FP)
            nc.gpsimd.memset(ones, 1.0)

            def transpose_scaled(src, tag, ps_bufs):
                """PE-transpose src [n,d] into [d,n] chunks, scale by 1/(K+1)
                on the PSUM->SBUF copy."""
                tps = ps_t_pool.tile(
                    [P, D], FP, tag=f"tps_{tag}", name=f"tps_{tag}", bufs=ps_bufs
                )
                for c in range(n_dc):
                    nc.tensor.transpose(
                        tps[:, bass.ts(c, P)], src[:, bass.ts(c, P)], ident
                    )
                t_sb = small_pool.tile(
                    [P, D], FP, tag=f"tsb_{tag}", name=f"tsb_{tag}",
                    bufs=(1 if tag == "b" else None),
                )
                nc.scalar.activation(
                    t_sb, tps, mybir.ActivationFunctionType.Copy, scale=INV
                )
                return t_sb

            def gemm_acc(out_pss, sumT, start):
                for c in range(n_dc):
                    for oh in range(len(out_pss)):
                        nc.tensor.matmul(
                            out_pss[oh],
                            lhsT=sumT[:, bass.ts(c, P)],
                            rhs=w_sb[:, c * O + oh * 512 : c * O + oh * 512 + 512],
                            start=(start and c == 0),
                            stop=False,
                        )

            k1 = (K + 1) // 2  # 13
            k2 = K - k1  # 12
            for i in range(nt):
                # split the neigh load so the k-sum (DVE tree adds; these run
                # at model speed where tensor_reduce measured ~1.6x slower)
                # starts while the second half streams, and SBUF slots
                # release at half-tile granularity
                nh1 = neigh_pool.tile([P, k1 * D], FP, tag="nh1", name="nh1")
                nc.sync.dma_start(nh1, neigh_h[bass.ts(i, P), 0:k1, :])
                nh2 = neigh_pool.tile([P, k2 * D], FP, tag="nh2", name="nh2")
                nc.sync.dma_start(nh2, neigh_h[bass.ts(i, P), k1:K, :])
                self_t = small_pool.tile([P, D], FP)
                nc.sync.dma_start(self_t, self_h[bass.ts(i, P), :])
                out_sb = out_pool.tile([P, O], FP)
                n_oh = O // 512
                out_pss = [
                    ps_o_pool.tile([P, 512], FP, tag=f"out_ps{oh}", name=f"out_ps{oh}")
                    for oh in range(n_oh)
                ]

                if i == nt - 1:
                    # tail tile: GEMM per half so half1's GEMM overlaps
                    # half2's DMA+tree and keeps the PE warm for half2's GEMM
                    _tree_fold(nc, nh1, k1)
                    summ1 = small_pool.tile([P, D], FP, tag="summ", name="summ1")
                    nc.vector.tensor_add(summ1, nh1[:, :D], self_t)
                    sumT1 = transpose_scaled(summ1, "a", None)
                    gemm_acc(out_pss, sumT1, start=True)
                    _tree_fold(nc, nh2, k2)
                    sumT2 = transpose_scaled(nh2[:, :D], "b", 1)
                    gemm_acc(out_pss, sumT2, start=False)
                else:
                    _tree_fold(nc, nh1, k1)
                    _tree_fold(nc, nh2, k2)
                    s12 = small_pool.tile([P, D], FP)
                    nc.vector.tensor_add(s12, nh1[:, :D], nh2[:, :D])
                    summ = small_pool.tile([P, D], FP)
                    nc.vector.tensor_add(summ, s12, self_t)
                    sumT = transpose_scaled(summ, "a", None)
                    gemm_acc(out_pss, sumT, start=True)

                for oh in range(n_oh):
                    # bias via K=1 matmul: ones.T @ b broadcasts b over nodes
                    nc.tensor.matmul(
                        out_pss[oh],
                        lhsT=ones,
                        rhs=b_sb[:, bass.ts(oh, 512)],
                        start=False,
                        stop=True,
                    )
                    nc.scalar.activation(
                        out_sb[:, bass.ts(oh, 512)],
                        out_pss[oh],
                        mybir.ActivationFunctionType.Relu,
                    )
                nc.scalar.dma_start(out_h[bass.ts(i, P), :], out_sb)

    nc.compile()
    return nc


def shard_inputs(inputs: dict) -> list[dict]:
    n = inputs["self_vecs"].shape[0]
    per = n // N_CORES
    maps = []
    for c in range(N_CORES):
        sl = slice(c * per, (c + 1) * per)
        maps.append(
            {
                "self_vecs": np.ascontiguousarray(inputs["self_vecs"][sl], np.float32),
                "neigh_vecs": np.ascontiguousarray(
                    inputs["neigh_vecs"][sl], np.float32
                ),
                "W": np.ascontiguousarray(inputs["W"], np.float32),
                "b": np.ascontiguousarray(inputs["b"], np.float32),
            }
        )
    return maps


def run_sharded(inputs: dict, trace: bool = False, **kwargs):
    from concourse.bass_utils import run_bass_kernel_spmd

    in_maps = shard_inputs(inputs)
    n_nodes = in_maps[0]["self_vecs"].shape[0]
    nc = build_nc(n_nodes)
    res = run_bass_kernel_spmd(
        nc, in_maps, core_ids=list(range(N_CORES)), trace=trace, **kwargs
    )
    out = np.concatenate([res.results[c]["out"] for c in range(N_CORES)], axis=0)
    return out, res


def kernel(**inputs) -> np.ndarray:
    out, _ = run_sharded(inputs, trace=False)
    return out
